# revision 1
# baseline (speedup 1.0000x reference)
"""GPTQ 4-bit quantized linear (CaiQuantLinear) on 8 TRN2 NeuronCores.

Computes out = x @ dequant(qweight, scales, qzeros) + bias where
  x: (4, 2048, 4096) fp16, qweight: (512, 4096) int32 (8x 4-bit per word,
  packed along input features), scales: (32, 4096) fp16, qzeros: (32, 512)
  int32 (packed along output features), bias: (4096,) fp16.
  Groups are contiguous blocks of 128 input features (g_idx = arange//128).

Sharding: tensor-parallel column split over output features. Each of the 8
cores gets 512 output columns (its slice of qweight/scales/qzeros/bias) and
the full x (replicated). No collectives; the host concatenates the 8 column
slices.

Per-core kernel:
  1. PE-transpose the raw int32 qweight words (bits ride through the fp16
     transpose path bitcast as fp32 — verified bit-exact on HW) into
     [out-feature, word-row] layout, where the nibble index varies along
     the free dim: immediate-shift unpack ops and a fused per-partition
     (subtract zero, multiply scale) tensor_scalar produce w^T fp16.
  2. PE-transpose w^T back to [input-feature, out] k-tiles, k8-major so
     k-tiles become ready in waves; fp16 weights stay resident in SBUF as
     [128, 32 k-tiles, 512 out]. Chunk-0 matmuls interleave with these
     waves using pre-allocated PSUM banks.
  3. Stream x through DMA-transpose (single HWDGE ring; two concurrent
     xbar rings corrupt data) into per-k [128, 1024-seq] tiles; k-major
     matmul order over 8 accumulating PSUM banks per chunk releases each
     xT tile after its 8 matmuls, keeping the transpose ring fed. Bias is
     added fp32 on the PSUM drain; output stores go via SWDGE.
"""

import sys

if "/opt/trn_rl_repo" not in sys.path:
    sys.path.insert(0, "/opt/trn_rl_repo")

import numpy as np

B, S, IN, OUT = 4, 2048, 4096, 4096
SEQ = B * S                      # 8192
NCORES = 8
OUT_S = OUT // NCORES            # 512 output columns per core
PACK = 8                         # int32 packs 8 nibbles
GSIZE = 128                      # group size == k-tile size

_CACHE = {}


def _build(seq, in_f, out_s, chunk):
    """Build + compile the per-core Bass program. All cores run the same
    NEFF on their own input slices (SPMD, no collectives)."""
    from contextlib import ExitStack  # noqa: F401

    import concourse.bass as bass  # noqa: F401
    import concourse.mybir as mybir
    import concourse.tile as tile
    from concourse import bacc
    from concourse.masks import make_identity

    dt = mybir.dt
    op = mybir.AluOpType
    P = 128
    KT = in_f // P                # k-tiles (== groups)
    QR = in_f // PACK             # qweight rows
    RT = QR // P                  # qweight row-tiles
    OT = out_s // P               # 128-wide output blocks per core
    NCH = seq // chunk            # seq chunks
    ST = chunk // P               # seq tiles per chunk

    nc = bacc.Bacc("TRN2", target_bir_lowering=False, debug=False,
                   num_devices=NCORES)

    x_d = nc.dram_tensor("x", (seq, in_f), dt.float16, kind="ExternalInput")
    qw_d = nc.dram_tensor("qweight", (QR, out_s), dt.int32, kind="ExternalInput")
    sc_d = nc.dram_tensor("scales", (KT, out_s), dt.float16, kind="ExternalInput")
    qz_d = nc.dram_tensor("qzeros", (KT, out_s // PACK), dt.int32,
                          kind="ExternalInput")
    b_d = nc.dram_tensor("bias", (1, out_s), dt.float16, kind="ExternalInput")
    out_d = nc.dram_tensor("out", (seq, out_s), dt.float16, kind="ExternalOutput")

    x = x_d.ap()
    qw = qw_d.ap()
    scales = sc_d.ap()
    qzeros = qz_d.ap()
    bias = b_d.ap()
    out = out_d.ap()

    with tile.TileContext(nc) as tc:
        with (
            tc.tile_pool(name="const", bufs=1) as const_pool,
            tc.tile_pool(name="w", bufs=1) as w_pool,
            tc.tile_pool(name="qst", bufs=4) as q_pool,
            tc.tile_pool(name="qt", bufs=4) as qt_pool,
            tc.tile_pool(name="wti", bufs=3) as wti_pool,
            tc.tile_pool(name="wt16", bufs=3) as wt16_pool,
            tc.tile_pool(name="bc", bufs=2) as bc_pool,
            tc.tile_pool(name="xt", bufs=44) as xt_pool,
            tc.tile_pool(name="ot", bufs=4) as out_pool,
            tc.tile_pool(name="ps", bufs=8, space="PSUM") as psum_pool,
            tc.tile_pool(name="dram", bufs=1, space="DRAM") as dram_pool,
        ):
            # ---- constants ----
            ident = const_pool.tile([P, P], dt.float16)
            make_identity(nc, ident)
            ident32 = const_pool.tile([P, P], dt.float32)
            make_identity(nc, ident32)

            bias16 = const_pool.tile([P, out_s], dt.float16)
            nc.gpsimd.dma_start(bias16, bias.to_broadcast((P, out_s)))
            bias32 = const_pool.tile([P, out_s], dt.float32)
            nc.vector.tensor_copy(bias32, bias16)

            # ---- dequantize weights ----
            # w_all[:, k, :]: k-tile k of fp16 weights, [128 in x out_s]
            w_all = w_pool.tile([P, KT, out_s], dt.float16)

            # chunk 0's xT tiles + 4 PSUM banks are set up before the
            # prologue so its first-half matmuls can interleave with the
            # per-row-tile dequant waves below.
            ST_H = min(4, ST)
            xts0 = []
            for k in range(KT):
                xtk0 = xt_pool.tile([P, chunk], dt.float16, tag="xt",
                                    name=f"xt0_{k}")
                nc.sync.dma_start(
                    xtk0, x[0:chunk, k * P:(k + 1) * P], transpose=True)
                xts0.append(xtk0)
            pss0 = [psum_pool.tile([P, out_s], dt.float32, tag="acc",
                                   name=f"ps0a_{st}") for st in range(ST_H)]

            # scales / zero-points in transposed layout: per 128-out block,
            # f32 per-partition columns indexed by group
            s_sb = const_pool.tile([KT, out_s], dt.float16)
            nc.gpsimd.dma_start(s_sb, scales)
            qz_sb = const_pool.tile([KT, out_s // PACK], dt.int32)
            nc.gpsimd.dma_start(qz_sb, qzeros)
            z_i = const_pool.tile([KT, out_s], dt.int32)
            z_iv = z_i.rearrange("g (c s) -> g c s", s=PACK)
            for s in range(PACK):
                nc.vector.tensor_scalar(
                    out=z_iv[:, :, s], in0=qz_sb, scalar1=4 * s, scalar2=0xF,
                    op0=op.logical_shift_right, op1=op.bitwise_and)
            z1_16 = const_pool.tile([KT, out_s], dt.float16)
            nc.vector.tensor_scalar_add(z1_16, z_i, 1.0)
            # transpose the small [KT, 128] scale/zero blocks on the PE
            z1T = []
            sT = []
            for ot in range(OT):
                pzs = psum_pool.tile([P, 2 * KT], dt.float16, tag="acc",
                                     name=f"pzs_{ot}")
                nc.tensor.transpose(
                    pzs[:, 0:KT],
                    z1_16[:, ot * P:(ot + 1) * P], ident[:KT, :KT])
                nc.tensor.transpose(
                    pzs[:, KT:2 * KT],
                    s_sb[:, ot * P:(ot + 1) * P], ident[:KT, :KT])
                zsf = const_pool.tile([P, 2 * KT], dt.float32,
                                      name=f"zsf_{ot}")
                nc.vector.tensor_copy(zsf, pzs)
                z1T.append(zsf[:, 0:KT])
                sT.append(zsf[:, KT:2 * KT])

            # transpose raw qweight words (int32 bits ride through the PE
            # transpose as fp32) -> qT[ot]: [128 out, 512 word-rows]
            q_nats = []
            for rt in range(RT):
                q_nat = q_pool.tile([P, out_s], dt.int32, tag="qnat")
                nc.gpsimd.dma_start(q_nat, qw[rt * P:(rt + 1) * P, :])
                q_nats.append(q_nat)
            qTs = []
            for ot in range(OT):
                pq = psum_pool.tile([P, QR], dt.float32, tag="acc",
                                    name=f"pq_{ot}")
                for rt in range(RT):
                    nc.tensor.transpose(
                        pq[:, rt * P:(rt + 1) * P],
                        q_nats[rt][:, ot * P:(ot + 1) * P].bitcast(
                            dt.float32),
                        ident32)
                qT = qt_pool.tile([P, QR], dt.int32, tag="qt",
                                  name=f"qT_{ot}")
                nc.vector.tensor_copy(qT, pq.bitcast(dt.int32))
                qTs.append(qT)

            # unpack + dequant + final transpose, k8-major so k-tiles
            # become ready in waves; chunk-0 first-half matmuls ride along
            NW = KT // PACK          # words / features per k8 wave
            for k8 in range(NW):
                for ot in range(OT):
                    # nibbles along the free dim: in-feature 8*wr + s
                    wT32 = wti_pool.tile([P, PACK * P], dt.int32, tag="wti")
                    wv = wT32.rearrange("p (w s) -> p w s", s=PACK)
                    w0 = k8 * (QR // NW)
                    for s in range(PACK):
                        nc.vector.tensor_scalar(
                            out=wv[:, :, s],
                            in0=qTs[ot][:, w0:w0 + QR // NW],
                            scalar1=4 * s, scalar2=0xF,
                            op0=op.logical_shift_right, op1=op.bitwise_and)
                    # fused dequant per group: (w - (z+1)) * scale -> fp16
                    wT16 = wt16_pool.tile([P, PACK * P], dt.float16,
                                          tag="wt16")
                    for gg in range(PACK):
                        g = k8 * PACK + gg
                        nc.vector.tensor_scalar(
                            out=wT16[:, gg * P:(gg + 1) * P],
                            in0=wT32[:, gg * P:(gg + 1) * P],
                            scalar1=z1T[ot][:, g:g + 1],
                            scalar2=sT[ot][:, g:g + 1],
                            op0=op.subtract, op1=op.mult)
                    pstB = psum_pool.tile([P, PACK * P], dt.float16,
                                          tag="acc")
                    for kk in range(PACK):
                        nc.tensor.transpose(
                            pstB[:, kk * P:(kk + 1) * P],
                            wT16[:, kk * P:(kk + 1) * P], ident)
                    nc.vector.tensor_copy(
                        w_all[:, k8 * PACK:(k8 + 1) * PACK,
                              ot * P:(ot + 1) * P],
                        pstB.rearrange("p (kk r) -> p kk r", r=P))
                # chunk-0 first-half matmuls for this wave's k-tiles
                for k in range(k8 * PACK, (k8 + 1) * PACK):
                    for st in range(ST_H):
                        nc.tensor.matmul(
                            pss0[st], lhsT=xts0[k][:, st * P:(st + 1) * P],
                            rhs=w_all[:, k, :],
                            start=(k == 0), stop=(k == KT - 1))

            # drain chunk-0 first half
            for st in range(ST_H):
                o16 = out_pool.tile([P, out_s], dt.float16, tag="o16",
                                    name=f"o16_0a_{st}")
                nc.vector.tensor_add(o16, pss0[st], bias32)
                nc.gpsimd.dma_start(out[st * P:(st + 1) * P, :], o16)
            # chunk-0 second half: fresh tiles so the first-half set frees
            # during the prologue instead of pinning 32 pool slots
            xts0b = []
            for k in range(KT):
                xtk0b = xt_pool.tile([P, chunk], dt.float16, tag="xt",
                                     name=f"xt0b_{k}")
                nc.sync.dma_start(
                    xtk0b, x[0:chunk, k * P:(k + 1) * P], transpose=True)
                xts0b.append(xtk0b)
            pss0b = [psum_pool.tile([P, out_s], dt.float32, tag="acc",
                                    name=f"ps0b_{st}")
                     for st in range(ST_H, ST)]
            for k in range(KT):
                for i, st in enumerate(range(ST_H, ST)):
                    nc.tensor.matmul(
                        pss0b[i], lhsT=xts0b[k][:, st * P:(st + 1) * P],
                        rhs=w_all[:, k, :],
                        start=(k == 0), stop=(k == KT - 1))
            for i, st in enumerate(range(ST_H, ST)):
                o16 = out_pool.tile([P, out_s], dt.float16, tag="o16",
                                    name=f"o16_0b_{st}")
                nc.vector.tensor_add(o16, pss0b[i], bias32)
                nc.gpsimd.dma_start(out[st * P:(st + 1) * P, :], o16)

            # ---- main loop: out[mseq, nout] = sum_k xT[k, m] * w[k, n] ----
            for cn in range(1, NCH):
                xts = []
                for k in range(KT):
                    xtk = xt_pool.tile([P, chunk], dt.float16, tag="xt")
                    nc.sync.dma_start(
                        xtk,
                        x[cn * chunk:(cn + 1) * chunk, k * P:(k + 1) * P],
                        transpose=True)
                    xts.append(xtk)
                pss = [psum_pool.tile([P, out_s], dt.float32, tag="acc",
                                      name=f"ps_{cn}_{st}")
                       for st in range(ST)]
                for k in range(KT):
                    for st in range(ST):
                        nc.tensor.matmul(
                            pss[st], lhsT=xts[k][:, st * P:(st + 1) * P],
                            rhs=w_all[:, k, :],
                            start=(k == 0), stop=(k == KT - 1))
                for st in range(ST):
                    o16 = out_pool.tile([P, out_s], dt.float16, tag="o16")
                    nc.vector.tensor_add(o16, pss[st], bias32)
                    r0 = cn * chunk + st * P
                    nc.gpsimd.dma_start(out[r0:r0 + P, :], o16)

    nc.compile()
    return nc


def _get_program(seq, in_f, out_s, chunk):
    key = (seq, in_f, out_s, chunk)
    if key not in _CACHE:
        _CACHE[key] = _build(seq, in_f, out_s, chunk)
    return _CACHE[key]


def kernel(x, qweight, scales, qzeros, g_idx=None, bias=None, **_unused):
    """Full-input entry point: shards over 8 cores, runs on HW, gathers."""
    from concourse.bass_utils import run_bass_kernel_spmd

    x = np.asarray(x)
    qweight = np.asarray(qweight)
    scales = np.asarray(scales)
    qzeros = np.asarray(qzeros)
    bias = np.asarray(bias)

    x2 = np.ascontiguousarray(x.reshape(SEQ, IN))
    nc = _get_program(SEQ, IN, OUT_S, 1024)

    zcols = OUT_S // PACK
    in_maps = []
    for c in range(NCORES):
        o0 = c * OUT_S
        in_maps.append({
            "x": x2,
            "qweight": np.ascontiguousarray(qweight[:, o0:o0 + OUT_S]),
            "scales": np.ascontiguousarray(scales[:, o0:o0 + OUT_S]),
            "qzeros": np.ascontiguousarray(qzeros[:, c * zcols:(c + 1) * zcols]),
            "bias": np.ascontiguousarray(bias[o0:o0 + OUT_S].reshape(1, OUT_S)),
        })

    res = run_bass_kernel_spmd(nc, in_maps, core_ids=list(range(NCORES)))
    full = np.concatenate([res.results[c]["out"] for c in range(NCORES)], axis=1)
    return full.reshape(B, S, OUT).astype(np.float16)



# revision 2
# speedup vs baseline: 1.3888x; 1.3888x over previous
"""GPTQ 4-bit quantized linear (CaiQuantLinear) on 8 TRN2 NeuronCores.

Computes out = x @ dequant(qweight, scales, qzeros) + bias where
  x: (4, 2048, 4096) fp16, qweight: (512, 4096) int32 (8x 4-bit per word,
  packed along input features), scales: (32, 4096) fp16, qzeros: (32, 512)
  int32 (packed along output features), bias: (4096,) fp16.
  Groups are contiguous blocks of 128 input features (g_idx = arange//128).

Sharding: tensor-parallel column split over output features. Each of the 8
cores gets 512 output columns (its slice of qweight/scales/qzeros/bias) and
the full x (replicated). No collectives; the host concatenates the 8 column
slices.

Host-side layout prep (pure data movement, no arithmetic): x is transposed
to [in, seq] so the device loads k-major lhsT tiles with large contiguous
packets instead of element-scatter transpose DMA; qweight rows are
replicated 8x so partition p of a k-tile holds the packed word for feature
p, unpacked in place with a per-partition shift.

Per-core kernel:
  1. Dequant (no PE involvement): unpack qzeros to a z+1 table [32, 512]
     int32, round-trip through DRAM so each group row can be broadcast to
     128 partitions. Per k-tile: natural load of the 8x-expanded qweight
     words, per-partition-shift unpack (vector), int32 subtract of the
     broadcast z1 row casting to fp16 (vector), fp16 multiply by the
     broadcast scale row (gpsimd) directly into the resident w_all
     [128, 32 k-tiles, 512 out] fp16.
  2. Matmul: 16 chunks of 512 seq positions; per chunk 4 PSUM banks
     accumulate over the 32 k-tiles (lhsT = xT 128x128 block, rhs = w_all
     k-slice 512 wide). Chunks ping-pong two 4-bank PSUM sets so the next
     chunk's matmuls overlap the previous drain. The first two chunks are
     interleaved k-wave by k-wave with the dequant so the PE starts
     immediately. Bias is added fp32 on the PSUM drain (vector); stores go
     via gpsimd SWDGE. x tiles stream on both HWDGE rings (sync/scalar)
     three chunks ahead.
"""

import sys

if "/opt/trn_rl_repo" not in sys.path:
    sys.path.insert(0, "/opt/trn_rl_repo")

import numpy as np

B, S, IN, OUT = 4, 2048, 4096, 4096
SEQ = B * S                      # 8192
NCORES = 8
OUT_S = OUT // NCORES            # 512 output columns per core
PACK = 8                         # int32 packs 8 nibbles
GSIZE = 128                      # group size == k-tile size
CHUNK = 512                      # seq positions per PSUM chunk

_CACHE = {}


def _build(seq, in_f, out_s, chunk):
    """Build + compile the per-core Bass program. All cores run the same
    NEFF on their own input slices (SPMD, no collectives)."""
    import concourse.bass as bass  # noqa: F401
    import concourse.mybir as mybir
    import concourse.tile as tile
    from concourse import bacc

    dt = mybir.dt
    op = mybir.AluOpType
    P = 128
    KT = in_f // P                # k-tiles (== groups) = 32
    CH = seq // chunk             # chunks = 16
    ST = chunk // P               # psum tiles per chunk = 4

    nc = bacc.Bacc("TRN2", target_bir_lowering=False, debug=False,
                   num_devices=NCORES)

    xT_d = nc.dram_tensor("xT", (in_f, seq), dt.float16, kind="ExternalInput")
    qb_d = nc.dram_tensor("qbig", (in_f, out_s), dt.int32,
                          kind="ExternalInput")
    sc_d = nc.dram_tensor("scales", (KT, out_s), dt.float16,
                          kind="ExternalInput")
    qz_d = nc.dram_tensor("qzeros", (KT, out_s // PACK), dt.int32,
                          kind="ExternalInput")
    b_d = nc.dram_tensor("bias", (1, out_s), dt.float16, kind="ExternalInput")
    sh_d = nc.dram_tensor("shifts", (P, 1), dt.int32, kind="ExternalInput")
    out_d = nc.dram_tensor("out", (seq, out_s), dt.float16,
                           kind="ExternalOutput")

    xT = xT_d.ap()
    qb = qb_d.ap()
    scales = sc_d.ap()
    qzeros = qz_d.ap()
    bias = b_d.ap()
    out = out_d.ap()

    with tile.TileContext(nc) as tc:
        with (
            tc.tile_pool(name="const", bufs=1) as const_pool,
            tc.tile_pool(name="w", bufs=1) as w_pool,
            tc.tile_pool(name="qk", bufs=3) as qk_pool,
            tc.tile_pool(name="zb", bufs=3) as zb_pool,
            tc.tile_pool(name="sb", bufs=3) as sb_pool,
            tc.tile_pool(name="wi", bufs=3) as wi_pool,
            tc.tile_pool(name="d16", bufs=3) as d_pool,
            tc.tile_pool(name="xt", bufs=96) as xt_pool,
            tc.tile_pool(name="ot", bufs=6) as out_pool,
            tc.tile_pool(name="ps", bufs=8, space="PSUM") as psum_pool,
            tc.tile_pool(name="dram", bufs=1, space="DRAM") as dram_pool,
        ):
            # ---- constants ----
            bias16 = const_pool.tile([P, out_s], dt.float16)
            nc.gpsimd.dma_start(bias16, bias.to_broadcast((P, out_s)))
            bias32 = const_pool.tile([P, out_s], dt.float32)
            nc.vector.tensor_copy(bias32, bias16)

            shifts = const_pool.tile([P, 1], dt.int32)
            nc.sync.dma_start(shifts, sh_d.ap())

            # ---- z+1 table: unpack qzeros, +1, round-trip to DRAM so the
            # per-group rows can be partition-broadcast ----
            qz_sb = const_pool.tile([KT, out_s // PACK], dt.int32)
            nc.sync.dma_start(qz_sb, qzeros)
            z_i = const_pool.tile([KT, out_s], dt.int32)
            z_iv = z_i.rearrange("g (c s) -> g c s", s=PACK)
            for s in range(PACK):
                nc.vector.tensor_scalar(
                    out=z_iv[:, :, s], in0=qz_sb, scalar1=4 * s, scalar2=0xF,
                    op0=op.logical_shift_right, op1=op.bitwise_and)
            ones = const_pool.tile([KT, out_s], dt.int32)
            nc.vector.memset(ones, 1)
            z1_i = const_pool.tile([KT, out_s], dt.int32)
            nc.vector.tensor_add(z1_i, z_i, ones)
            z1_d = dram_pool.tile([KT, out_s], dt.int32)
            nc.gpsimd.dma_start(z1_d, z1_i)

            # fp16 weights stay resident: w_all[:, k, :] is k-tile k
            w_all = w_pool.tile([P, KT, out_s], dt.float16)

            # ---- x streaming / matmul helpers ----
            xts = {}

            def load_chunk(c):
                lst = []
                for k in range(KT):
                    t = xt_pool.tile([P, chunk], dt.float16, tag="xt",
                                     name=f"xt_{c}_{k}")
                    eng = nc.sync if (k % 2 == 0) else nc.scalar
                    eng.dma_start(
                        t, xT[k * P:(k + 1) * P, c * chunk:(c + 1) * chunk])
                    lst.append(t)
                xts[c] = lst

            pss = {}

            def alloc_ps(c):
                pss[c] = [psum_pool.tile([P, out_s], dt.float32, tag="acc",
                                         name=f"ps_{c}_{st}")
                          for st in range(ST)]

            def mm(c, k):
                for st in range(ST):
                    nc.tensor.matmul(
                        pss[c][st],
                        lhsT=xts[c][k][:, st * P:(st + 1) * P],
                        rhs=w_all[:, k, :],
                        start=(k == 0), stop=(k == KT - 1))

            def drain(c):
                for st in range(ST):
                    o16 = out_pool.tile([P, out_s], dt.float16, tag="o16",
                                        name=f"o16_{c}_{st}")
                    nc.vector.tensor_add(o16, pss[c][st], bias32)
                    r0 = c * chunk + st * P
                    nc.gpsimd.dma_start(out[r0:r0 + P, :], o16)
                del pss[c]
                del xts[c]

            # ---- dequant loop, interleaved with chunks 0 and 1 ----
            load_chunk(0)
            load_chunk(1)
            alloc_ps(0)
            alloc_ps(1)
            for k in range(KT):
                qk = qk_pool.tile([P, out_s], dt.int32, tag="qk")
                nc.sync.dma_start(qk, qb[k * P:(k + 1) * P, :])
                z1bc = zb_pool.tile([P, out_s], dt.int32, tag="zb")
                nc.scalar.dma_start(
                    z1bc, z1_d[k:k + 1, :].to_broadcast((P, out_s)))
                sbc = sb_pool.tile([P, out_s], dt.float16, tag="sb")
                nc.scalar.dma_start(
                    sbc, scales[k:k + 1, :].to_broadcast((P, out_s)))
                wi32 = wi_pool.tile([P, out_s], dt.int32, tag="wi")
                nc.vector.tensor_scalar(
                    out=wi32, in0=qk, scalar1=shifts, scalar2=0xF,
                    op0=op.logical_shift_right, op1=op.bitwise_and)
                d16 = d_pool.tile([P, out_s], dt.float16, tag="d16")
                nc.vector.tensor_tensor(
                    out=d16, in0=wi32, in1=z1bc, op=op.subtract)
                nc.gpsimd.tensor_mul(w_all[:, k, :], d16, sbc)
                mm(0, k)
                mm(1, k)
            drain(0)
            drain(1)

            # ---- steady-state chunks ----
            for c in range(2, CH):
                load_chunk(c)
                alloc_ps(c)
                for k in range(KT):
                    mm(c, k)
                drain(c)

    nc.compile()
    return nc


def _get_program(seq, in_f, out_s, chunk):
    key = (seq, in_f, out_s, chunk)
    if key not in _CACHE:
        _CACHE[key] = _build(seq, in_f, out_s, chunk)
    return _CACHE[key]


def _make_in_maps(x, qweight, scales, qzeros, bias):
    """Host-side sharding + layout prep shared by kernel() and test.py."""
    x2 = np.asarray(x).reshape(SEQ, IN)
    xT = np.ascontiguousarray(x2.T)                      # [IN, SEQ]
    qweight = np.asarray(qweight)
    scales = np.asarray(scales)
    qzeros = np.asarray(qzeros)
    bias = np.asarray(bias)
    sh = ((np.arange(128) % PACK) * 4).astype(np.int32).reshape(128, 1)

    zcols = OUT_S // PACK
    in_maps = []
    for c in range(NCORES):
        o0 = c * OUT_S
        in_maps.append({
            "xT": xT,
            "qbig": np.ascontiguousarray(
                np.repeat(qweight[:, o0:o0 + OUT_S], PACK, axis=0)),
            "scales": np.ascontiguousarray(scales[:, o0:o0 + OUT_S]),
            "qzeros": np.ascontiguousarray(
                qzeros[:, c * zcols:(c + 1) * zcols]),
            "bias": np.ascontiguousarray(
                bias[o0:o0 + OUT_S].reshape(1, OUT_S)),
            "shifts": sh,
        })
    return in_maps


def kernel(x, qweight, scales, qzeros, g_idx=None, bias=None, **_unused):
    """Full-input entry point: shards over 8 cores, runs on HW, gathers."""
    from concourse.bass_utils import run_bass_kernel_spmd

    nc = _get_program(SEQ, IN, OUT_S, CHUNK)
    in_maps = _make_in_maps(x, qweight, scales, qzeros, bias)

    res = run_bass_kernel_spmd(nc, in_maps, core_ids=list(range(NCORES)))
    full = np.concatenate([res.results[c]["out"] for c in range(NCORES)],
                          axis=1)
    return full.reshape(B, S, OUT).astype(np.float16)


# revision 3
# speedup vs baseline: 1.5043x; 1.0832x over previous
"""GPTQ 4-bit quantized linear (CaiQuantLinear) on 8 TRN2 NeuronCores.

Computes out = x @ dequant(qweight, scales, qzeros) + bias where
  x: (4, 2048, 4096) fp16, qweight: (512, 4096) int32 (8x 4-bit per word,
  packed along input features), scales: (32, 4096) fp16, qzeros: (32, 512)
  int32 (packed along output features), bias: (4096,) fp16.
  Groups are contiguous blocks of 128 input features (g_idx = arange//128).

Sharding: tensor-parallel column split over output features. Each of the 8
cores gets 512 output columns (its slice of qweight/scales/qzeros/bias) and
the full x (replicated). No collectives; the host concatenates the 8 column
slices.

Host-side layout prep (pure data movement, no arithmetic): x is transposed
to [in, seq] so the device loads k-major lhsT tiles with large contiguous
packets instead of element-scatter transpose DMA; qweight rows are
replicated 8x so partition p of a k-tile holds the packed word for feature
p, unpacked in place with a per-partition shift.

Per-core kernel:
  1. Dequant (no PE involvement): unpack qzeros to a z+1 table [32, 512]
     int32, round-trip through DRAM so each group row can be broadcast to
     128 partitions. Per k-tile: natural load of the 8x-expanded qweight
     words, per-partition-shift unpack (vector), int32 subtract of the
     broadcast z1 row casting to fp16 (vector), fp16 multiply by the
     broadcast scale row (gpsimd) directly into the resident w_all
     [128, 32 k-tiles, 512 out] fp16.
  2. Matmul: 16 chunks of 512 seq positions; per chunk 4 PSUM banks
     accumulate over the 32 k-tiles (lhsT = xT 128x128 block, rhs = w_all
     k-slice 512 wide). Chunks ping-pong two 4-bank PSUM sets so the next
     chunk's matmuls overlap the previous drain. The first two chunks are
     interleaved k-wave by k-wave with the dequant so the PE starts
     immediately. Bias is added fp32 on the PSUM drain (vector); stores go
     via gpsimd SWDGE. x tiles stream on both HWDGE rings (sync/scalar)
     three chunks ahead.
"""

import sys

if "/opt/trn_rl_repo" not in sys.path:
    sys.path.insert(0, "/opt/trn_rl_repo")

import numpy as np

B, S, IN, OUT = 4, 2048, 4096, 4096
SEQ = B * S                      # 8192
NCORES = 8
OUT_S = OUT // NCORES            # 512 output columns per core
PACK = 8                         # int32 packs 8 nibbles
GSIZE = 128                      # group size == k-tile size
CHUNK = 512                      # seq positions per PSUM chunk

_CACHE = {}


def _build(seq, in_f, out_s, chunk):
    """Build + compile the per-core Bass program. All cores run the same
    NEFF on their own input slices (SPMD, no collectives)."""
    import concourse.bass as bass  # noqa: F401
    import concourse.mybir as mybir
    import concourse.tile as tile
    from concourse import bacc

    dt = mybir.dt
    op = mybir.AluOpType
    P = 128
    KT = in_f // P                # k-tiles (== groups) = 32
    CH = seq // chunk             # chunks = 16
    ST = chunk // P               # psum tiles per chunk = 4

    nc = bacc.Bacc("TRN2", target_bir_lowering=False, debug=False,
                   num_devices=NCORES)

    xT_d = nc.dram_tensor("xT", (in_f, seq), dt.float16, kind="ExternalInput")
    qb_d = nc.dram_tensor("qbig", (in_f, out_s), dt.int32,
                          kind="ExternalInput")
    sc_d = nc.dram_tensor("scales", (KT, out_s), dt.float16,
                          kind="ExternalInput")
    qz_d = nc.dram_tensor("qzeros", (KT, out_s // PACK), dt.int32,
                          kind="ExternalInput")
    b_d = nc.dram_tensor("bias", (1, out_s), dt.float16, kind="ExternalInput")
    sh_d = nc.dram_tensor("shifts", (P, 1), dt.int32, kind="ExternalInput")
    out_d = nc.dram_tensor("out", (seq, out_s), dt.float16,
                           kind="ExternalOutput")

    xT = xT_d.ap()
    qb = qb_d.ap()
    scales = sc_d.ap()
    qzeros = qz_d.ap()
    bias = b_d.ap()
    out = out_d.ap()

    with tile.TileContext(nc) as tc:
        with (
            tc.tile_pool(name="const", bufs=1) as const_pool,
            tc.tile_pool(name="w", bufs=1) as w_pool,
            tc.tile_pool(name="qk", bufs=3) as qk_pool,
            tc.tile_pool(name="zb", bufs=3) as zb_pool,
            tc.tile_pool(name="sb", bufs=3) as sb_pool,
            tc.tile_pool(name="wi", bufs=3) as wi_pool,
            tc.tile_pool(name="d16", bufs=3) as d_pool,
            tc.tile_pool(name="xt", bufs=96) as xt_pool,
            tc.tile_pool(name="ot", bufs=6) as out_pool,
            tc.tile_pool(name="ps", bufs=8, space="PSUM") as psum_pool,
            tc.tile_pool(name="dram", bufs=1, space="DRAM") as dram_pool,
        ):
            # ---- constants ----
            bias16 = const_pool.tile([P, out_s], dt.float16)
            nc.gpsimd.dma_start(bias16, bias.to_broadcast((P, out_s)))
            bias32 = const_pool.tile([P, out_s], dt.float32)
            nc.vector.tensor_copy(bias32, bias16)

            shifts = const_pool.tile([P, 1], dt.int32)
            nc.sync.dma_start(shifts, sh_d.ap())

            # ---- z+1 table: unpack qzeros, +1, round-trip to DRAM so the
            # per-group rows can be partition-broadcast ----
            qz_sb = const_pool.tile([KT, out_s // PACK], dt.int32)
            nc.sync.dma_start(qz_sb, qzeros)
            z_i = const_pool.tile([KT, out_s], dt.int32)
            z_iv = z_i.rearrange("g (c s) -> g c s", s=PACK)
            for s in range(PACK):
                nc.vector.tensor_scalar(
                    out=z_iv[:, :, s], in0=qz_sb, scalar1=4 * s, scalar2=0xF,
                    op0=op.logical_shift_right, op1=op.bitwise_and)
            ones = const_pool.tile([KT, out_s], dt.int32)
            nc.vector.memset(ones, 1)
            z1_i = const_pool.tile([KT, out_s], dt.int32)
            nc.vector.tensor_add(z1_i, z_i, ones)
            z1_d = dram_pool.tile([KT, out_s], dt.int32)
            nc.gpsimd.dma_start(z1_d, z1_i)

            # fp16 weights stay resident: w_all[:, k, :] is k-tile k
            w_all = w_pool.tile([P, KT, out_s], dt.float16)

            # ---- x streaming / matmul helpers ----
            xts = {}

            def load_chunk(c):
                lst = []
                for k in range(KT):
                    t = xt_pool.tile([P, chunk], dt.float16, tag="xt",
                                     name=f"xt_{c}_{k}")
                    eng = nc.sync if (k % 2 == 0) else nc.scalar
                    eng.dma_start(
                        t, xT[k * P:(k + 1) * P, c * chunk:(c + 1) * chunk])
                    lst.append(t)
                xts[c] = lst

            pss = {}

            def alloc_ps(c):
                pss[c] = [psum_pool.tile([P, out_s], dt.float32, tag="acc",
                                         name=f"ps_{c}_{st}")
                          for st in range(ST)]

            def mm(c, k):
                for st in range(ST):
                    nc.tensor.matmul(
                        pss[c][st],
                        lhsT=xts[c][k][:, st * P:(st + 1) * P],
                        rhs=w_all[:, k, :],
                        start=(k == 0), stop=(k == KT - 1))

            def drain(c):
                for st in range(ST):
                    o16 = out_pool.tile([P, out_s], dt.float16, tag="o16",
                                        name=f"o16_{c}_{st}")
                    nc.vector.tensor_add(o16, pss[c][st], bias32)
                    r0 = c * chunk + st * P
                    nc.gpsimd.dma_start(out[r0:r0 + P, :], o16)
                del pss[c]
                del xts[c]

            # ---- dequant loop, interleaved with chunks 0 and 1 ----
            # Per k-wave the dequant DMAs go at the ring head (in-order
            # rings: x tiles queued ahead would starve the dequant chain),
            # the k's own x tiles ride just behind, and the byte load is
            # mirrored by k parity so both HWDGE rings carry equal traffic.
            xts[0] = []
            xts[1] = []
            alloc_ps(0)
            alloc_ps(1)
            for k in range(KT):
                ea = nc.sync if k % 2 == 0 else nc.scalar
                eb = nc.scalar if k % 2 == 0 else nc.sync
                qk = qk_pool.tile([P, out_s], dt.int32, tag="qk")
                ea.dma_start(qk, qb[k * P:(k + 1) * P, :])
                z1bc = zb_pool.tile([P, out_s], dt.int32, tag="zb")
                eb.dma_start(
                    z1bc, z1_d[k:k + 1, :].to_broadcast((P, out_s)))
                sbc = sb_pool.tile([P, out_s], dt.float16, tag="sb")
                eb.dma_start(
                    sbc, scales[k:k + 1, :].to_broadcast((P, out_s)))
                for c in (0, 1):
                    t = xt_pool.tile([P, chunk], dt.float16, tag="xt",
                                     name=f"xt_{c}_{k}")
                    (ea if c == 0 else eb).dma_start(
                        t, xT[k * P:(k + 1) * P, c * chunk:(c + 1) * chunk])
                    xts[c].append(t)
                wi32 = wi_pool.tile([P, out_s], dt.int32, tag="wi")
                nc.vector.tensor_scalar(
                    out=wi32, in0=qk, scalar1=shifts, scalar2=0xF,
                    op0=op.logical_shift_right, op1=op.bitwise_and)
                d16 = d_pool.tile([P, out_s], dt.float16, tag="d16")
                nc.vector.tensor_tensor(
                    out=d16, in0=wi32, in1=z1bc, op=op.subtract)
                nc.gpsimd.tensor_mul(w_all[:, k, :], d16, sbc)
                mm(0, k)
                mm(1, k)
            drain(0)
            drain(1)

            # ---- steady-state chunks ----
            for c in range(2, CH):
                load_chunk(c)
                alloc_ps(c)
                for k in range(KT):
                    mm(c, k)
                drain(c)

    nc.compile()
    return nc


def _get_program(seq, in_f, out_s, chunk):
    key = (seq, in_f, out_s, chunk)
    if key not in _CACHE:
        _CACHE[key] = _build(seq, in_f, out_s, chunk)
    return _CACHE[key]


def _make_in_maps(x, qweight, scales, qzeros, bias):
    """Host-side sharding + layout prep shared by kernel() and test.py."""
    x2 = np.asarray(x).reshape(SEQ, IN)
    xT = np.ascontiguousarray(x2.T)                      # [IN, SEQ]
    qweight = np.asarray(qweight)
    scales = np.asarray(scales)
    qzeros = np.asarray(qzeros)
    bias = np.asarray(bias)
    sh = ((np.arange(128) % PACK) * 4).astype(np.int32).reshape(128, 1)

    zcols = OUT_S // PACK
    in_maps = []
    for c in range(NCORES):
        o0 = c * OUT_S
        in_maps.append({
            "xT": xT,
            "qbig": np.ascontiguousarray(
                np.repeat(qweight[:, o0:o0 + OUT_S], PACK, axis=0)),
            "scales": np.ascontiguousarray(scales[:, o0:o0 + OUT_S]),
            "qzeros": np.ascontiguousarray(
                qzeros[:, c * zcols:(c + 1) * zcols]),
            "bias": np.ascontiguousarray(
                bias[o0:o0 + OUT_S].reshape(1, OUT_S)),
            "shifts": sh,
        })
    return in_maps


def kernel(x, qweight, scales, qzeros, g_idx=None, bias=None, **_unused):
    """Full-input entry point: shards over 8 cores, runs on HW, gathers."""
    from concourse.bass_utils import run_bass_kernel_spmd

    nc = _get_program(SEQ, IN, OUT_S, CHUNK)
    in_maps = _make_in_maps(x, qweight, scales, qzeros, bias)

    res = run_bass_kernel_spmd(nc, in_maps, core_ids=list(range(NCORES)))
    full = np.concatenate([res.results[c]["out"] for c in range(NCORES)],
                          axis=1)
    return full.reshape(B, S, OUT).astype(np.float16)
